# revision 4
# baseline (speedup 1.0000x reference)
"""GQA attention (B=4,S=1024,D=2048,H=32,KVH=8,HD=64) + RoPE, tensor-parallel
over the 8 kv-head groups across 8 NeuronCores.

v2 changes vs v1 baseline:
  - out-proj packs head PAIRS: avn_pair [128, S] (even head rows 0:64, odd
    64:128) against stacked Wo rows -> K=128 matmuls (halves out-proj PE time)
  - odd-head AV uses v_aug_o [tokens,128] (ones col 0, v cols 64:128) so its
    avT lands at partitions 64:128 directly (denominator at row 0)
  - 1/denom broadcast via K=1 ones-matmul on PE (+1 copy) instead of the
    DRAM-roundtrip + gpsimd broadcast DMA per head
  - out-proj(b) emission interleaved with scores/exp of (b+1, pr=0) so ACT
    stays fed and score matmuls fill yps copy stalls
  - y output in bf16 (halves output DMA)
"""

import numpy as np
import ml_dtypes

import concourse.bass as bass
import concourse.mybir as mybir
import concourse.tile as tile
from concourse import bacc
from concourse import bass_utils

BF16 = mybir.dt.bfloat16
F32 = mybir.dt.float32
BF = ml_dtypes.bfloat16

B, S, D = 4, 1024, 2048
H, KVH, HD = 32, 8, 64
NREP = H // KVH          # 4 q heads per core
T = B * S                # 4096 tokens
NC = 8                   # cores
QD = NREP * HD           # 256 q dims per core
KC = D // 128            # 16 contraction chunks
TB = 512                 # proj token-block
NTB = T // TB            # 8
AF = mybir.ActivationFunctionType

_CACHE = {}


def _build():
    key = "nc"
    if key in _CACHE:
        return _CACHE[key]
    nc = bacc.Bacc("TRN2", target_bir_lowering=False)
    # Pin all ACT table lookups to set 6 (natural_log_exp_and_others: has
    # Exp, Ln, Copy) so the kernel needs exactly one table load.
    import concourse.bacc as _bacc_mod
    _orig_tables = _bacc_mod.get_activation_tables

    def _pinned_tables(arch):
        items = list(_orig_tables(arch).items())
        return {k: (v if i == 6 else set()) for i, (k, v) in enumerate(items)}

    _bacc_mod.get_activation_tables = _pinned_tables

    xT_d = nc.dram_tensor("xT", (D, T), BF16, kind="ExternalInput")
    wq_d = nc.dram_tensor("wq", (D, QD), BF16, kind="ExternalInput")
    wkv_d = nc.dram_tensor("wkv", (D, 128), BF16, kind="ExternalInput")
    wo_d = nc.dram_tensor("wo", (128, 2 * D), BF16, kind="ExternalInput")
    cos_d = nc.dram_tensor("cos2", (128, S), F32, kind="ExternalInput")
    sin_d = nc.dram_tensor("sin2", (128, S), F32, kind="ExternalInput")
    arot_d = nc.dram_tensor("arot", (128, 128), BF16, kind="ExternalInput")
    eye_d = nc.dram_tensor("eye64", (64, 64), BF16, kind="ExternalInput")
    y_d = nc.dram_tensor("y", (T, D), BF16, kind="ExternalOutput")

    with tile.TileContext(nc) as tc:
        with (
            tc.tile_pool(name="const", bufs=1) as cpool,
            tc.tile_pool(name="persist", bufs=1) as ppool,
        ):
            # ---- constants ----
            wq_sb = cpool.tile([128, KC * QD], BF16, tag="wq")
            wq_dv = wq_d[:].rearrange("(c p) m -> p c m", p=128)
            wq_sv = wq_sb[:].rearrange("p (c m) -> p c m", c=KC)
            nc.sync.dma_start(out=wq_sv[:, 0:4, :], in_=wq_dv[:, 0:4, :])
            wkv_sb = cpool.tile([128, KC * 128], BF16, tag="wkv")
            nc.sync.dma_start(
                out=wkv_sb[:].rearrange("p (c m) -> p c m", c=KC),
                in_=wkv_d[:].rearrange("(c p) m -> p c m", p=128),
            )
            nc.sync.dma_start(out=wq_sv[:, 4:KC, :], in_=wq_dv[:, 4:KC, :])
            # Wo packed per head-pair: wo2[:, pr*D + n] rows 0:64 head 2pr,
            # rows 64:128 head 2pr+1 (host preps [128, 2*D])
            wo2_sb = cpool.tile([128, 2 * D], BF16, tag="wo")
            nc.sync.dma_start(out=wo2_sb[:], in_=wo_d[:])
            cos_sb = cpool.tile([128, S], F32, tag="cos")
            nc.sync.dma_start(out=cos_sb[:], in_=cos_d[:])
            sin_sb = cpool.tile([128, S], F32, tag="sin")
            nc.sync.dma_start(out=sin_sb[:], in_=sin_d[:])
            arot_sb = cpool.tile([128, 128], BF16, tag="arot")
            nc.sync.dma_start(out=arot_sb[:], in_=arot_d[:])
            eye_sb = cpool.tile([64, 64], BF16, tag="eye")
            nc.sync.dma_start(out=eye_sb[:], in_=eye_d[:])
            ones_sb = cpool.tile([1, 128], BF16, tag="ones")
            nc.vector.memset(ones_sb[:], 1.0)

            # ---- persistent activations ----
            qrope = [ppool.tile([128, T], BF16, tag=f"qrope{p}", name=f"qrope{p}") for p in range(2)]
            kT2 = ppool.tile([128, T], BF16, tag="kT2")
            vT_sb = ppool.tile([128, T], BF16, tag="vT")      # rows 64:128 used
            vtmpT = ppool.tile([64, T], BF16, tag="vtmpT")
            v_aug_e = [ppool.tile([128, 8 * 65], BF16, tag=f"vauge{b}", name=f"vauge{b}") for b in range(B)]
            v_aug_o = [ppool.tile([128, 8 * 128], BF16, tag=f"vaugo{b}", name=f"vaugo{b}") for b in range(B)]
            for b in range(B):
                vo = v_aug_o[b][:].rearrange("p (k c) -> p k c", k=8)
                nc.vector.memset(vo[:, :, 0:1], 1.0)
                nc.vector.memset(vo[:, :, 1:64], 0.0)

            # ================= projection phase =================
            with (
                tc.tile_pool(name="xin", bufs=2) as xpool,
                tc.tile_pool(name="rtmp", bufs=3) as rpool,
                tc.tile_pool(name="pj", bufs=1, space="PSUM") as pj,
                tc.tile_pool(name="pshift", bufs=2, space="PSUM") as psh,
                tc.tile_pool(name="pvtr", bufs=2, space="PSUM") as pvt,
            ):
                for tb in range(NTB):
                    b, scol = tb // 2, (tb % 2) * TB
                    tcols = bass.ts(tb, TB)
                    xts = xpool.tile([128, KC * TB], BF16, tag="xts")
                    nc.sync.dma_start(
                        out=xts[:].rearrange("p (c n) -> p c n", c=KC),
                        in_=xT_d[:, tcols].rearrange("(c p) n -> p c n", p=128),
                    )
                    q0ps = pj.tile([128, TB], F32, tag="q0")
                    q1ps = pj.tile([128, TB], F32, tag="q1")
                    kvps = pj.tile([128, TB], F32, tag="kv")
                    for c in range(KC):
                        xc = xts[:, bass.ts(c, TB)]
                        st = dict(start=(c == 0), stop=(c == KC - 1))
                        nc.tensor.matmul(q0ps[:], wq_sb[:, c * QD:c * QD + 128], xc, **st)
                        nc.tensor.matmul(q1ps[:], wq_sb[:, c * QD + 128:(c + 1) * QD], xc, **st)
                        nc.tensor.matmul(kvps[:], wkv_sb[:, bass.ts(c, 128)], xc, **st)
                    css, sns = cos_sb[:, scol:scol + TB], sin_sb[:, scol:scol + TB]
                    # q pairs RoPE
                    for p, qps in ((0, q0ps), (1, q1ps)):
                        qsin = rpool.tile([128, TB], BF16, tag="qsin")
                        nc.vector.tensor_mul(qsin[:], qps[:], sns)
                        t1 = rpool.tile([128, TB], F32, tag="t1")
                        nc.vector.tensor_mul(t1[:], qps[:], css)
                        shift = psh.tile([128, TB], F32, tag="shift")
                        nc.tensor.matmul(shift[:], arot_sb[:], qsin[:], start=True, stop=True)
                        nc.vector.tensor_add(qrope[p][:, tcols], t1[:], shift[:])
                    # k RoPE on rows 0:64
                    ksin = rpool.tile([64, TB], BF16, tag="qsin")
                    nc.vector.tensor_mul(ksin[:], kvps[0:64, :], sns[0:64])
                    t1k = rpool.tile([64, TB], F32, tag="t1")
                    nc.vector.tensor_mul(t1k[:], kvps[0:64, :], css[0:64])
                    shk = psh.tile([128, TB], F32, tag="shift")
                    nc.tensor.matmul(shk[0:64, :], arot_sb[0:64, 0:64], ksin[:], start=True, stop=True)
                    nc.vector.tensor_add(kT2[0:64, tcols], t1k[:], shk[0:64, :])
                    # v: copy to rows 64:128, then DMA down to partitions 0:64
                    nc.scalar.copy(vT_sb[64:128, tcols], kvps[64:128, :])
                    nc.sync.dma_start(out=vtmpT[:, tcols], in_=vT_sb[64:128, tcols])
                    if tb % 2 == 1:
                        # batch b complete: build v natural layouts
                        for kb in range(8):
                            vtr = pvt.tile([128, 64], BF16, tag="vtr")
                            nc.tensor.transpose(
                                vtr[:], vtmpT[:, b * S + kb * 128:b * S + (kb + 1) * 128],
                                eye_sb[:],
                            )
                            nc.scalar.copy(v_aug_e[b][:, kb * 65:kb * 65 + 64], vtr[:])
                            nc.scalar.copy(v_aug_o[b][:, kb * 128 + 64:(kb + 1) * 128], vtr[:])
                        nc.vector.memset(
                            v_aug_e[b][:].rearrange("p (k o) -> p k o", k=8)[:, :, 64:65], 1.0
                        )
                # duplicate k_rope to rows 64:128 (for head-odd row tiling)
                nc.sync.dma_start(out=kT2[64:128, :], in_=kT2[0:64, :])

            # ================= attention + output phase =================
            with (
                tc.tile_pool(name="prob", bufs=2) as prpool,
                tc.tile_pool(name="inv", bufs=2) as ipool,
                tc.tile_pool(name="rbcs", bufs=2) as rpool2,
                tc.tile_pool(name="avns", bufs=4) as apool,
                tc.tile_pool(name="yout", bufs=2) as ypool,
                tc.tile_pool(name="ps_s", bufs=2, space="PSUM") as pss,
                tc.tile_pool(name="ps_av", bufs=2, space="PSUM") as psa,
            ):
                avn = [[None, None] for _ in range(B)]
                probs = {}

                def scores_kb(b, pr, kb):
                    """Score matmuls + exp for one 128-key block of (b, pr)."""
                    prob0, prob1 = probs[(b, pr)]
                    sps0 = pss.tile([128, S], F32, tag="s", name="sps0")
                    sps1 = pss.tile([128, S], F32, tag="s", name="sps1")
                    kcol = slice(b * S + kb * 128, b * S + (kb + 1) * 128)
                    for qh in range(2):
                        qcol = slice(b * S + qh * 512, b * S + (qh + 1) * 512)
                        nc.tensor.matmul(
                            sps0[:, bass.ts(qh, 512)], kT2[0:64, kcol],
                            qrope[pr][0:64, qcol], start=True, stop=True)
                        nc.tensor.matmul(
                            sps1[:, bass.ts(qh, 512)], kT2[64:128, kcol],
                            qrope[pr][64:128, qcol], start=True, stop=True)
                    nc.scalar.activation(prob0[:, bass.ts(kb, S)], sps0[:], AF.Exp, scale=0.125)
                    nc.scalar.activation(prob1[:, bass.ts(kb, S)], sps1[:], AF.Exp, scale=0.125)

                def av_kb(b, pr, kb, avps_e, avps_o):
                    prob0, prob1 = probs[(b, pr)]
                    st = dict(start=(kb == 0), stop=(kb == 7))
                    for qh in range(2):
                        pcol = slice(kb * S + qh * 512, kb * S + (qh + 1) * 512)
                        nc.tensor.matmul(
                            avps_e[0:65, bass.ts(qh, 512)],
                            v_aug_e[b][:, kb * 65:(kb + 1) * 65], prob0[:, pcol], **st)
                        nc.tensor.matmul(
                            avps_o[:, bass.ts(qh, 512)],
                            v_aug_o[b][:, bass.ts(kb, 128)], prob1[:, pcol], **st)

                def normalize(b, pr, avps_e, avps_o):
                    """1/denom via ln->exp(-x); broadcast via K=1 matmul."""
                    lnt = ipool.tile([1, S], F32, tag="lnt")
                    inv_e = ipool.tile([1, S], BF16, tag="inv_e")
                    nc.scalar.activation(lnt[:], avps_e[64:65, :], AF.Ln)
                    nc.scalar.activation(inv_e[:], lnt[:], AF.Exp, scale=-1.0)
                    lnt2 = ipool.tile([1, S], F32, tag="lnt2")
                    inv_o = ipool.tile([1, S], BF16, tag="inv_o")
                    nc.scalar.activation(lnt2[:], avps_o[0:1, :], AF.Ln)
                    nc.scalar.activation(inv_o[:], lnt2[:], AF.Exp, scale=-1.0)
                    rbc_ps = pss.tile([128, S], F32, tag="s", name="rbc")
                    for qh in range(2):
                        nc.tensor.matmul(
                            rbc_ps[0:64, bass.ts(qh, 512)], ones_sb[0:1, 0:64],
                            inv_e[:, bass.ts(qh, 512)], start=True, stop=True)
                        nc.tensor.matmul(
                            rbc_ps[64:128, bass.ts(qh, 512)], ones_sb[0:1, 64:128],
                            inv_o[:, bass.ts(qh, 512)], start=True, stop=True)
                    rbc_sb = rpool2.tile([128, S], BF16, tag="rbc_sb")
                    nc.vector.tensor_copy(rbc_sb[:], rbc_ps[:])
                    avn_t = apool.tile([128, S], BF16, tag="avn")
                    nc.vector.tensor_mul(avn_t[0:64, :], avps_e[0:64, :], rbc_sb[0:64, :])
                    nc.vector.tensor_mul(avn_t[64:128, :], avps_o[64:128, :], rbc_sb[64:128, :])
                    avn[b][pr] = avn_t

                def attention(b, pr, interleave=None):
                    """Full attention for (b, pr). If interleave is given, it
                    is a list of thunks (out-proj yps groups of the PREVIOUS
                    batch) to emit between score blocks."""
                    probs[(b, pr)] = (
                        prpool.tile([128, 8 * S], BF16, tag="prob", name="prob0"),
                        prpool.tile([128, 8 * S], BF16, tag="prob", name="prob1"),
                    )
                    if interleave is None:
                        avps_e = psa.tile([128, S], F32, tag="av", name="avps_e")
                        avps_o = psa.tile([128, S], F32, tag="av", name="avps_o")
                        for kb in range(8):
                            scores_kb(b, pr, kb)
                            av_kb(b, pr, kb, avps_e, avps_o)
                        normalize(b, pr, avps_e, avps_o)
                    else:
                        # scores/exp first (interleaved with prev out-proj);
                        # AV afterwards once avps slots free up.
                        for kb in range(8):
                            if kb < len(interleave):
                                interleave[kb]()
                            scores_kb(b, pr, kb)
                        for th in interleave[8:]:
                            th()
                        avps_e = psa.tile([128, S], F32, tag="av", name="avps_e")
                        avps_o = psa.tile([128, S], F32, tag="av", name="avps_o")
                        for kb in range(8):
                            av_kb(b, pr, kb, avps_e, avps_o)
                        normalize(b, pr, avps_e, avps_o)

                def outproj_thunks(b):
                    """Out-proj for batch b as 8 thunks (one per token chunk)."""
                    def mk(t):
                        def th():
                            ysb = ypool.tile([128, D], BF16, tag="ysb")
                            for half in range(2):
                                yps = psa.tile([128, S], F32, tag="av", name="yps")
                                for sub in range(2):
                                    nb = half * 2 + sub
                                    for pr in range(2):
                                        nc.tensor.matmul(
                                            yps[:, bass.ts(sub, 512)],
                                            avn[b][pr][:, bass.ts(t, 128)],
                                            wo2_sb[:, pr * D + nb * 512:pr * D + (nb + 1) * 512],
                                            start=(pr == 0), stop=(pr == 1))
                                nc.vector.tensor_copy(ysb[:, bass.ts(half, 1024)], yps[:])
                            nc.sync.dma_start(
                                out=y_d[b * S + t * 128:b * S + (t + 1) * 128, :],
                                in_=ysb[:])
                        return th
                    return [mk(t) for t in range(8)]

                attention(0, 0)
                attention(0, 1)
                for b in range(B):
                    thunks = outproj_thunks(b)
                    if b + 1 < B:
                        attention(b + 1, 0, interleave=thunks)
                        attention(b + 1, 1)
                    else:
                        for th in thunks:
                            th()

    try:
        nc.compile()
    finally:
        _bacc_mod.get_activation_tables = _orig_tables
    _CACHE[key] = nc
    return nc


def _host_prep(x, cos, sin, Wq, Wk, Wv, Wo):
    x = np.asarray(x, np.float32)
    xT = np.ascontiguousarray(x.reshape(T, D).T).astype(BF)
    cosT = np.asarray(cos, np.float32).T
    sinT = np.asarray(sin, np.float32).T
    cos2 = np.ascontiguousarray(np.tile(cosT, (2, 1)))          # (128, S) f32
    sin2 = np.ascontiguousarray(np.tile(sinT, (2, 1)))
    # lhsT for qshiftT = A @ qT  ->  arot = A.T (block-diag x2 over heads)
    A = np.zeros((HD, HD), np.float32)
    for d in range(32):
        A[d, d + 32] = -1.0
        A[32 + d, d] = 1.0
    arot = np.kron(np.eye(2, dtype=np.float32), A.T).astype(BF)  # (128,128)
    eye64 = np.eye(64, dtype=np.float32).astype(BF)

    Wq = np.asarray(Wq, np.float32)
    Wk = np.asarray(Wk, np.float32)
    Wv = np.asarray(Wv, np.float32)
    Wo = np.asarray(Wo, np.float32)
    in_maps = []
    for g in range(NC):
        wq_g = np.ascontiguousarray(Wq[:, g * QD:(g + 1) * QD]).astype(BF)
        wkv_g = np.ascontiguousarray(
            np.concatenate([Wk[:, g * HD:(g + 1) * HD], Wv[:, g * HD:(g + 1) * HD]], axis=1)
        ).astype(BF)
        # head-pair packing: [128, 2*D], pr block = Wo_g rows pr*128:(pr+1)*128
        wo_g = np.ascontiguousarray(
            Wo[g * QD:(g + 1) * QD, :].reshape(2, 128, D).transpose(1, 0, 2).reshape(128, 2 * D)
        ).astype(BF)
        in_maps.append({
            "xT": xT, "wq": wq_g, "wkv": wkv_g, "wo": wo_g,
            "cos2": cos2, "sin2": sin2, "arot": arot, "eye64": eye64,
        })
    return in_maps


def kernel(x, cos, sin, Wq, Wk, Wv, Wo):
    nc = _build()
    in_maps = _host_prep(x, cos, sin, Wq, Wk, Wv, Wo)
    res = bass_utils.run_bass_kernel_spmd(
        nc, in_maps, core_ids=list(range(NC)), trace=False,
    )
    y = np.zeros((T, D), np.float32)
    for r in res.results:
        y += np.asarray(r["y"], np.float32)
    return y.reshape(B, S, D)


# revision 8
# speedup vs baseline: 1.9257x; 1.9257x over previous
"""GQA attention (B=4,S=1024,D=2048,H=32,KVH=8,HD=64) + RoPE, tensor-parallel
over the 8 kv-head groups across 8 NeuronCores.

v2 changes vs v1 baseline:
  - out-proj packs head PAIRS: avn_pair [128, S] (even head rows 0:64, odd
    64:128) against stacked Wo rows -> K=128 matmuls (halves out-proj PE time)
  - odd-head AV uses v_aug_o [tokens,128] (ones col 0, v cols 64:128) so its
    avT lands at partitions 64:128 directly (denominator at row 0)
  - 1/denom broadcast via K=1 ones-matmul on PE (+1 copy) instead of the
    DRAM-roundtrip + gpsimd broadcast DMA per head
  - out-proj(b) emission interleaved with scores/exp of (b+1, pr=0) so ACT
    stays fed and score matmuls fill yps copy stalls
  - y output in bf16 (halves output DMA)
"""

import numpy as np
import ml_dtypes

import concourse.bass as bass
import concourse.mybir as mybir
import concourse.tile as tile
from concourse import bacc
from concourse import bass_utils

BF16 = mybir.dt.bfloat16
F32 = mybir.dt.float32
BF = ml_dtypes.bfloat16

B, S, D = 4, 1024, 2048
H, KVH, HD = 32, 8, 64
NREP = H // KVH          # 4 q heads per core
T = B * S                # 4096 tokens
NC = 8                   # cores
QD = NREP * HD           # 256 q dims per core
KC = D // 128            # 16 contraction chunks
TB = 512                 # proj token-block
NTB = T // TB            # 8
AF = mybir.ActivationFunctionType

_CACHE = {}


def _build():
    key = "nc"
    if key in _CACHE:
        return _CACHE[key]
    nc = bacc.Bacc("TRN2", target_bir_lowering=False)
    # Pin all ACT table lookups to set 6 (natural_log_exp_and_others: has
    # Exp, Ln, Copy) so the kernel needs exactly one table load.
    import concourse.bacc as _bacc_mod
    _orig_tables = _bacc_mod.get_activation_tables

    def _pinned_tables(arch):
        items = list(_orig_tables(arch).items())
        return {k: (v if i == 6 else set()) for i, (k, v) in enumerate(items)}

    _bacc_mod.get_activation_tables = _pinned_tables

    xT_d = nc.dram_tensor("xT", (D, T), BF16, kind="ExternalInput")
    wq_d = nc.dram_tensor("wq", (D, QD), BF16, kind="ExternalInput")
    wkv_d = nc.dram_tensor("wkv", (D, 128), BF16, kind="ExternalInput")
    wo_d = nc.dram_tensor("wo", (128, 2 * D), BF16, kind="ExternalInput")
    cos_d = nc.dram_tensor("cos2", (128, S), F32, kind="ExternalInput")
    sin_d = nc.dram_tensor("sin2", (128, S), F32, kind="ExternalInput")
    arot_d = nc.dram_tensor("arot", (128, 128), BF16, kind="ExternalInput")
    eye_d = nc.dram_tensor("eye64", (64, 64), BF16, kind="ExternalInput")
    y_d = nc.dram_tensor("y", (T, D), BF16, kind="ExternalOutput")

    with tile.TileContext(nc) as tc:
        with (
            tc.tile_pool(name="const", bufs=1) as cpool,
            tc.tile_pool(name="persist", bufs=1) as ppool,
        ):
            # ---- constants (first-needed-first: wq/wkv chunk 0:2 now, the
            # rest after the first x tile is queued below) ----
            wq_sb = cpool.tile([128, KC * QD], BF16, tag="wq")
            wq_dv = wq_d[:].rearrange("(c p) m -> p c m", p=128)
            wq_sv = wq_sb[:].rearrange("p (c m) -> p c m", c=KC)
            nc.sync.dma_start(out=wq_sv[:, 0:2, :], in_=wq_dv[:, 0:2, :])
            wkv_sb = cpool.tile([128, KC * 128], BF16, tag="wkv")
            wkv_dv = wkv_d[:].rearrange("(c p) m -> p c m", p=128)
            wkv_sv = wkv_sb[:].rearrange("p (c m) -> p c m", c=KC)
            nc.sync.dma_start(out=wkv_sv[:, 0:2, :], in_=wkv_dv[:, 0:2, :])
            wo2_sb = cpool.tile([128, 2 * D], BF16, tag="wo")
            cos_sb = cpool.tile([128, S], F32, tag="cos")
            sin_sb = cpool.tile([128, S], F32, tag="sin")
            arot_sb = cpool.tile([128, 128], BF16, tag="arot")
            eye_sb = cpool.tile([64, 64], BF16, tag="eye")
            ones_sb = cpool.tile([1, 128], BF16, tag="ones")
            nc.vector.memset(ones_sb[:], 1.0)

            # ---- persistent activations ----
            qrope = [ppool.tile([128, T], BF16, tag=f"qrope{p}", name=f"qrope{p}") for p in range(2)]
            kT2 = ppool.tile([128, T], BF16, tag="kT2")
            vT_sb = ppool.tile([128, T], BF16, tag="vT")      # rows 64:128 used
            vtmpT = ppool.tile([64, T], BF16, tag="vtmpT")
            v_aug_e = [ppool.tile([128, 8 * 65], BF16, tag=f"vauge{b}", name=f"vauge{b}") for b in range(B)]
            v_aug_o = [ppool.tile([128, 8 * 128], BF16, tag=f"vaugo{b}", name=f"vaugo{b}") for b in range(B)]
            for b in range(B):
                vo = v_aug_o[b][:].rearrange("p (k c) -> p k c", k=8)
                nc.vector.memset(vo[:, :, 0:1], 1.0)
                nc.vector.memset(vo[:, :, 1:64], 0.0)

            # ================= projection phase =================
            with (
                tc.tile_pool(name="xin", bufs=2) as xpool,
                tc.tile_pool(name="rtmp", bufs=3) as rpool,
                tc.tile_pool(name="pj", bufs=1, space="PSUM") as pj,
                tc.tile_pool(name="pshift", bufs=2, space="PSUM") as psh,
                tc.tile_pool(name="pvtr", bufs=2, space="PSUM") as pvt,
            ):
                # first x tile: queue its leading chunks before bulky consts
                xts0 = xpool.tile([128, KC * TB], BF16, tag="xts")
                xts0_v = xts0[:].rearrange("p (c n) -> p c n", c=KC)
                x0_dv = xT_d[:, bass.ts(0, TB)].rearrange("(c p) n -> p c n", p=128)
                nc.sync.dma_start(out=xts0_v[:, 0:2, :], in_=x0_dv[:, 0:2, :])
                nc.sync.dma_start(out=wq_sv[:, 2:KC, :], in_=wq_dv[:, 2:KC, :])
                nc.sync.dma_start(out=wkv_sv[:, 2:KC, :], in_=wkv_dv[:, 2:KC, :])
                nc.sync.dma_start(out=xts0_v[:, 2:KC, :], in_=x0_dv[:, 2:KC, :])
                nc.sync.dma_start(out=cos_sb[:], in_=cos_d[:])
                nc.sync.dma_start(out=sin_sb[:], in_=sin_d[:])
                nc.sync.dma_start(out=arot_sb[:], in_=arot_d[:])
                nc.sync.dma_start(out=eye_sb[:], in_=eye_d[:])
                nc.sync.dma_start(out=wo2_sb[:], in_=wo_d[:])
                for tb in range(NTB):
                    b, scol = tb // 2, (tb % 2) * TB
                    tcols = bass.ts(tb, TB)
                    if tb == 0:
                        xts = xts0
                    else:
                        xts = xpool.tile([128, KC * TB], BF16, tag="xts")
                        nc.sync.dma_start(
                            out=xts[:].rearrange("p (c n) -> p c n", c=KC),
                            in_=xT_d[:, tcols].rearrange("(c p) n -> p c n", p=128),
                        )
                    q0ps = pj.tile([128, TB], F32, tag="q0")
                    q1ps = pj.tile([128, TB], F32, tag="q1")
                    kvps = pj.tile([128, TB], F32, tag="kv")
                    for c in range(KC):
                        xc = xts[:, bass.ts(c, TB)]
                        st = dict(start=(c == 0), stop=(c == KC - 1))
                        nc.tensor.matmul(q0ps[:], wq_sb[:, c * QD:c * QD + 128], xc, **st)
                        nc.tensor.matmul(q1ps[:], wq_sb[:, c * QD + 128:(c + 1) * QD], xc, **st)
                        nc.tensor.matmul(kvps[:], wkv_sb[:, bass.ts(c, 128)], xc, **st)
                    css, sns = cos_sb[:, scol:scol + TB], sin_sb[:, scol:scol + TB]
                    # q pairs RoPE
                    for p, qps in ((0, q0ps), (1, q1ps)):
                        qsin = rpool.tile([128, TB], BF16, tag="qsin")
                        nc.vector.tensor_mul(qsin[:], qps[:], sns)
                        t1 = rpool.tile([128, TB], F32, tag="t1")
                        nc.vector.tensor_mul(t1[:], qps[:], css)
                        shift = psh.tile([128, TB], F32, tag="shift")
                        nc.tensor.matmul(shift[:], arot_sb[:], qsin[:], start=True, stop=True)
                        nc.vector.tensor_add(qrope[p][:, tcols], t1[:], shift[:])
                    # k RoPE on rows 0:64
                    ksin = rpool.tile([64, TB], BF16, tag="qsin")
                    nc.vector.tensor_mul(ksin[:], kvps[0:64, :], sns[0:64])
                    t1k = rpool.tile([64, TB], F32, tag="t1")
                    nc.vector.tensor_mul(t1k[:], kvps[0:64, :], css[0:64])
                    shk = psh.tile([128, TB], F32, tag="shift")
                    nc.tensor.matmul(shk[0:64, :], arot_sb[0:64, 0:64], ksin[:], start=True, stop=True)
                    nc.vector.tensor_add(kT2[0:64, tcols], t1k[:], shk[0:64, :])
                    # v: copy to rows 64:128, then DMA down to partitions 0:64
                    nc.scalar.copy(vT_sb[64:128, tcols], kvps[64:128, :])
                    nc.sync.dma_start(out=vtmpT[:, tcols], in_=vT_sb[64:128, tcols])
                    if tb % 2 == 1:
                        # batch b complete: build v natural layouts
                        for kb in range(8):
                            vtr = pvt.tile([128, 64], BF16, tag="vtr")
                            nc.tensor.transpose(
                                vtr[:], vtmpT[:, b * S + kb * 128:b * S + (kb + 1) * 128],
                                eye_sb[:],
                            )
                            nc.scalar.copy(v_aug_e[b][:, kb * 65:kb * 65 + 64], vtr[:])
                            nc.scalar.copy(v_aug_o[b][:, kb * 128 + 64:(kb + 1) * 128], vtr[:])
                        nc.vector.memset(
                            v_aug_e[b][:].rearrange("p (k o) -> p k o", k=8)[:, :, 64:65], 1.0
                        )
                # duplicate k_rope to rows 64:128 (for head-odd row tiling)
                nc.sync.dma_start(out=kT2[64:128, :], in_=kT2[0:64, :])

            # ================= attention + output phase =================
            with (
                tc.tile_pool(name="prob", bufs=2) as prpool,
                tc.tile_pool(name="inv", bufs=2) as ipool,
                tc.tile_pool(name="rbcs", bufs=2) as rpool2,
                tc.tile_pool(name="avns", bufs=4) as apool,
                tc.tile_pool(name="yout", bufs=2) as ypool,
                tc.tile_pool(name="ps_s", bufs=2, space="PSUM") as pss,
                tc.tile_pool(name="ps_av", bufs=2, space="PSUM") as psa,
            ):
                avn = [[None, None] for _ in range(B)]
                probs = {}

                def scores_kb(b, pr, kb):
                    """Score matmuls + exp for one 128-key block of (b, pr)."""
                    prob0, prob1 = probs[(b, pr)]
                    sps0 = pss.tile([128, S], F32, tag="s", name="sps0")
                    sps1 = pss.tile([128, S], F32, tag="s", name="sps1")
                    kcol = slice(b * S + kb * 128, b * S + (kb + 1) * 128)
                    for qh in range(2):
                        qcol = slice(b * S + qh * 512, b * S + (qh + 1) * 512)
                        nc.tensor.matmul(
                            sps0[:, bass.ts(qh, 512)], kT2[0:64, kcol],
                            qrope[pr][0:64, qcol], start=True, stop=True)
                        nc.tensor.matmul(
                            sps1[:, bass.ts(qh, 512)], kT2[64:128, kcol],
                            qrope[pr][64:128, qcol], start=True, stop=True)
                    nc.scalar.activation(prob0[:, bass.ts(kb, S)], sps0[:], AF.Exp, scale=0.125)
                    nc.scalar.activation(prob1[:, bass.ts(kb, S)], sps1[:], AF.Exp, scale=0.125)

                def av_kb(b, pr, kb, avps_e, avps_o):
                    prob0, prob1 = probs[(b, pr)]
                    st = dict(start=(kb == 0), stop=(kb == 7))
                    for qh in range(2):
                        pcol = slice(kb * S + qh * 512, kb * S + (qh + 1) * 512)
                        nc.tensor.matmul(
                            avps_e[0:65, bass.ts(qh, 512)],
                            v_aug_e[b][:, kb * 65:(kb + 1) * 65], prob0[:, pcol], **st)
                        nc.tensor.matmul(
                            avps_o[:, bass.ts(qh, 512)],
                            v_aug_o[b][:, bass.ts(kb, 128)], prob1[:, pcol], **st)

                def normalize(b, pr, avps_e, avps_o):
                    """1/denom via ln->exp(-x); broadcast via K=1 matmul."""
                    lnt = ipool.tile([1, S], F32, tag="lnt")
                    inv_e = ipool.tile([1, S], BF16, tag="inv_e")
                    nc.scalar.activation(lnt[:], avps_e[64:65, :], AF.Ln)
                    nc.scalar.activation(inv_e[:], lnt[:], AF.Exp, scale=-1.0)
                    lnt2 = ipool.tile([1, S], F32, tag="lnt2")
                    inv_o = ipool.tile([1, S], BF16, tag="inv_o")
                    nc.scalar.activation(lnt2[:], avps_o[0:1, :], AF.Ln)
                    nc.scalar.activation(inv_o[:], lnt2[:], AF.Exp, scale=-1.0)
                    rbc_ps = pss.tile([128, S], F32, tag="s", name="rbc")
                    for qh in range(2):
                        nc.tensor.matmul(
                            rbc_ps[0:64, bass.ts(qh, 512)], ones_sb[0:1, 0:64],
                            inv_e[:, bass.ts(qh, 512)], start=True, stop=True)
                        nc.tensor.matmul(
                            rbc_ps[64:128, bass.ts(qh, 512)], ones_sb[0:1, 64:128],
                            inv_o[:, bass.ts(qh, 512)], start=True, stop=True)
                    rbc_sb = rpool2.tile([128, S], BF16, tag="rbc_sb")
                    nc.vector.tensor_copy(rbc_sb[:], rbc_ps[:])
                    avn_t = apool.tile([128, S], BF16, tag="avn")
                    nc.vector.tensor_mul(avn_t[0:64, :], avps_e[0:64, :], rbc_sb[0:64, :])
                    nc.vector.tensor_mul(avn_t[64:128, :], avps_o[64:128, :], rbc_sb[64:128, :])
                    avn[b][pr] = avn_t

                def attention(b, pr, interleave=None):
                    """Full attention for (b, pr). If interleave is given, it
                    is a list of thunks (out-proj yps groups of the PREVIOUS
                    batch) to emit between score blocks."""
                    probs[(b, pr)] = (
                        prpool.tile([128, 8 * S], BF16, tag="prob", name="prob0"),
                        prpool.tile([128, 8 * S], BF16, tag="prob", name="prob1"),
                    )
                    if interleave is None:
                        avps_e = psa.tile([128, S], F32, tag="av", name="avps_e")
                        avps_o = psa.tile([128, S], F32, tag="av", name="avps_o")
                        for kb in range(8):
                            scores_kb(b, pr, kb)
                            av_kb(b, pr, kb, avps_e, avps_o)
                        normalize(b, pr, avps_e, avps_o)
                    else:
                        # scores/exp first (interleaved with prev out-proj);
                        # AV afterwards once avps slots free up.
                        for kb in range(8):
                            if kb < len(interleave):
                                interleave[kb]()
                            scores_kb(b, pr, kb)
                        for th in interleave[8:]:
                            th()
                        avps_e = psa.tile([128, S], F32, tag="av", name="avps_e")
                        avps_o = psa.tile([128, S], F32, tag="av", name="avps_o")
                        for kb in range(8):
                            av_kb(b, pr, kb, avps_e, avps_o)
                        normalize(b, pr, avps_e, avps_o)

                def outproj_thunks(b):
                    """Out-proj for batch b as 16 thunks (one yps group each;
                    two consecutive thunks share a ysb token-chunk)."""
                    ysb_box = [None]

                    def mk(t, half):
                        def th():
                            if half == 0:
                                ysb_box[0] = ypool.tile([128, D], BF16, tag="ysb", name="ysb")
                            ysb = ysb_box[0]
                            yps = psa.tile([128, S], F32, tag="av", name="yps")
                            for sub in range(2):
                                nb = half * 2 + sub
                                for pr in range(2):
                                    nc.tensor.matmul(
                                        yps[:, bass.ts(sub, 512)],
                                        avn[b][pr][:, bass.ts(t, 128)],
                                        wo2_sb[:, pr * D + nb * 512:pr * D + (nb + 1) * 512],
                                        start=(pr == 0), stop=(pr == 1))
                            nc.vector.tensor_copy(ysb[:, bass.ts(half, 1024)], yps[:])
                            if half == 1:
                                nc.sync.dma_start(
                                    out=y_d[b * S + t * 128:b * S + (t + 1) * 128, :],
                                    in_=ysb[:])
                        return th
                    return [mk(t, half) for t in range(8) for half in range(2)]

                attention(0, 0)
                attention(0, 1)
                for b in range(B):
                    thunks = outproj_thunks(b)
                    if b + 1 < B:
                        attention(b + 1, 0, interleave=thunks[0:8])
                        attention(b + 1, 1, interleave=thunks[8:16])
                    else:
                        for th in thunks:
                            th()

    try:
        nc.compile()
    finally:
        _bacc_mod.get_activation_tables = _orig_tables
    _CACHE[key] = nc
    return nc


def _host_prep(x, cos, sin, Wq, Wk, Wv, Wo):
    x = np.asarray(x, np.float32)
    xT = np.ascontiguousarray(x.reshape(T, D).T).astype(BF)
    cosT = np.asarray(cos, np.float32).T
    sinT = np.asarray(sin, np.float32).T
    cos2 = np.ascontiguousarray(np.tile(cosT, (2, 1)))          # (128, S) f32
    sin2 = np.ascontiguousarray(np.tile(sinT, (2, 1)))
    # lhsT for qshiftT = A @ qT  ->  arot = A.T (block-diag x2 over heads)
    A = np.zeros((HD, HD), np.float32)
    for d in range(32):
        A[d, d + 32] = -1.0
        A[32 + d, d] = 1.0
    arot = np.kron(np.eye(2, dtype=np.float32), A.T).astype(BF)  # (128,128)
    eye64 = np.eye(64, dtype=np.float32).astype(BF)

    Wq = np.asarray(Wq, np.float32)
    Wk = np.asarray(Wk, np.float32)
    Wv = np.asarray(Wv, np.float32)
    Wo = np.asarray(Wo, np.float32)
    in_maps = []
    for g in range(NC):
        wq_g = np.ascontiguousarray(Wq[:, g * QD:(g + 1) * QD]).astype(BF)
        wkv_g = np.ascontiguousarray(
            np.concatenate([Wk[:, g * HD:(g + 1) * HD], Wv[:, g * HD:(g + 1) * HD]], axis=1)
        ).astype(BF)
        # head-pair packing: [128, 2*D], pr block = Wo_g rows pr*128:(pr+1)*128
        wo_g = np.ascontiguousarray(
            Wo[g * QD:(g + 1) * QD, :].reshape(2, 128, D).transpose(1, 0, 2).reshape(128, 2 * D)
        ).astype(BF)
        in_maps.append({
            "xT": xT, "wq": wq_g, "wkv": wkv_g, "wo": wo_g,
            "cos2": cos2, "sin2": sin2, "arot": arot, "eye64": eye64,
        })
    return in_maps


def kernel(x, cos, sin, Wq, Wk, Wv, Wo):
    nc = _build()
    in_maps = _host_prep(x, cos, sin, Wq, Wk, Wv, Wo)
    res = bass_utils.run_bass_kernel_spmd(
        nc, in_maps, core_ids=list(range(NC)), trace=False,
    )
    y = np.zeros((T, D), np.float32)
    for r in res.results:
        y += np.asarray(r["y"], np.float32)
    return y.reshape(B, S, D)
